# revision 54
# baseline (speedup 1.0000x reference)
"""GINEConv (2-layer, N=100k, E=1.6M, H=128, G=64) on 8 Trainium2 cores.

Single SPMD launch. Nodes + incident (dst) edges partitioned across 8
cores; edges dst-sorted and tiled into 128-dst windows; per-window
aggregation is one-hot scatter-matmuls accumulating in PSUM. Layer-1
messages are expanded on-device from (x_src, edge_attr, 1) streams via a
K=3 matmul + ReLU. h1 rows are written node-major (bf16) to DRAM, then
AllGather'd across the 8 cores; layer-2 per-edge h1[src] rows come from
bulk dma_gather calls (4 per tile: slots grouped by src_row mod 4 so
int16 class-local indices address a 1024B-stride view of the table),
then a DVE add of the edge-encoder term + relu. Pooling is a per-core
one-hot matmul + AllReduce; the classifier runs redundantly on every
core.
"""

import json
import os

try:
    import orjson as _fastjson
except ImportError:
    _fastjson = None

import ml_dtypes
import numpy as np

import concourse.bass as bass
import concourse.bass2jax as _b2j
import concourse.bass_utils as _bu
import concourse.tile as tile
from concourse import library_config, library_overlay, mybir
from concourse.bass_utils import run_bass_kernel_spmd

F32 = mybir.dt.float32
BF16 = mybir.dt.bfloat16
I32 = mybir.dt.int32
I16 = mybir.dt.int16
RELU = mybir.ActivationFunctionType.Relu
SIG = mybir.ActivationFunctionType.Sigmoid
COPY = mybir.ActivationFunctionType.Copy
ADD = mybir.AluOpType.add
MULT = mybir.AluOpType.mult
MAX = mybir.AluOpType.max
ISEQ = mybir.AluOpType.is_equal

N, E, H, G = 100000, 1600000, 128, 64
NCORES = 8
NLOC = N // NCORES            # 12500 nodes per core
NPAD = 12544                  # padded to 98*128
NT = NPAD // 128              # 98 node tiles per core
BN_EPS = 1e-5
bf = ml_dtypes.bfloat16

# ---------------------------------------------------------------- fixups


_WS_CTR = [0]


def _split_multiwait_bir(bir_json):
    data = bir_json.decode() if isinstance(bir_json, (bytes, bytearray)) else bir_json
    bir = _fastjson.loads(data) if _fastjson else json.loads(data)
    changed = False
    for f in bir.get("functions", []):
        for b in f.get("blocks", []):
            out = []
            for inst in b.get("instructions", []):
                si = inst.get("sync_info") or {}
                waits = si.get("on_wait") or []
                if len(waits) > 1:
                    changed = True
                    for w in waits[:-1]:
                        _WS_CTR[0] += 1
                        nop = {
                            "name": f"I-wsplit-{_WS_CTR[0]}",
                            "opcode": "NoOp",
                            "engine": inst["engine"],
                            "ins": [],
                            "outs": [],
                            "sync_info": {"on_update": [], "on_wait": [w]},
                        }
                        if "debug" in inst:
                            nop["debug"] = inst["debug"]
                        out.append(nop)
                    si["on_wait"] = [waits[-1]]
                out.append(inst)
            b["instructions"] = out
    if not changed:
        return bir_json
    return _fastjson.dumps(bir) if _fastjson else json.dumps(bir).encode()


_ORIG_COMPILE = _bu.compile_bir_kernel


def _patched_compile(bir_json, *args, **kwargs):
    return _ORIG_COMPILE(_split_multiwait_bir(bir_json), *args, **kwargs)


def _install_fixups():
    _bu.compile_bir_kernel = _patched_compile
    _b2j.compile_bir_kernel = _patched_compile


# ---------------------------------------------------------------- host prep


QB = np.array([0, 4096, 8192, 11520], np.int64)
QS = np.array([4096, 4096, 3328, 1024], np.int64)


def _edge_meta(src, dst):
    """Cheap prefix: dst-sort with src-quarter class grouping per (core, tile).

    Classes are src-node quarters (by local row range) so the layer-2 gather
    uses int16 table-local indices into one of four AllGather'd quarter
    tables. Columns are laid out per SUPERTILE (pair of dst tiles),
    class-major, so each (supertile, class) is one contiguous gather call.
    """
    core = dst // NLOC
    dloc = dst - core * NLOC
    tile_id = dloc // 128
    src_core = src // NLOC
    src_loc = src - src_core * NLOC
    cls = np.searchsorted(QB[1:], src_loc, side="right")
    order = np.lexsort((dloc, cls, tile_id, core))
    core_s, tile_s, cls_s, dloc_s = (
        core[order], tile_id[order], cls[order], dloc[order],
    )

    counts = np.zeros((NCORES, NT, 4), np.int64)
    np.add.at(counts, (core_s, tile_s, cls_s), 1)
    chcls = (counts.max(axis=0) + 127) // 128          # [NT, 4] chunk cols
    chunks = chcls.sum(axis=1)                         # [NT]
    chunks2 = chunks[0::2] + chunks[1::2]              # [NS] supertile cols
    col_off2 = np.concatenate([[0], chunks2.cumsum()])[:-1]
    totcols = int(chunks2.sum())
    chq = chcls[0::2] + chcls[1::2]                    # [NS, 4]
    cls_off2 = np.zeros((NT // 2, 4), np.int64)
    cls_off2[:, 1:] = chq.cumsum(axis=1)[:, :-1]
    ngrp2 = (chunks2 + 3) // 4
    sgrp_off = np.concatenate([[0], ngrp2.cumsum()])[:-1]
    totgrp = int(ngrp2.sum())

    starts = np.zeros(NCORES * NT * 4, np.int64)
    flat = counts.reshape(-1).cumsum()
    starts[1:] = flat[:-1]
    return (order, core_s, tile_s, cls_s, dloc_s, starts, chunks, chcls,
            chq, cls_off2, col_off2, ngrp2, sgrp_off, totcols, totgrp)


def _edge_fill(src, ea, x, order, core_s, tile_s, cls_s, dloc_s, starts,
               chcls, cls_off2, col_off2, sgrp_off, totcols, totgrp, maxgrp2):
    """Heavy per-core slot streams; run in a worker thread.

    Supertile s covers dst tiles (2s, 2s+1); its columns are class-major:
    [cls0: t-even chunks, t-odd chunks][cls1: ...]. Slot i of a
    (supertile, class) gather call -> partition i%128, chunk-col i//128
    within the class block.
    Returns per-core dict:
      sloc  bf16 [128, totcols] dst index within its own tile (-1 pads)
      p3g   bf16 [12, (totgrp+maxgrp2)*128] grouped (x, ea, 1) stream per
            supertile (groups of 4 consecutive supertile columns)
      idx16 i16  [128, 8*totcols] quarter-table gather indices packed
            i%16 / i//16, replicated 8x across partitions
    """
    src_s = src[order]
    ea_s = ea[order]
    src_core = src_s // NLOC
    src_loc = src_s - src_core * NLOC
    q16 = (src_core * QS[cls_s] + (src_loc - QB[cls_s])).astype(np.int16)

    run_id = (core_s * NT + tile_s) * 4 + cls_s
    rank = np.arange(len(src_s), dtype=np.int64) - starts[run_id]
    sgrp = tile_s // 2
    par = tile_s % 2
    i_call = par * chcls[tile_s - par, cls_s] * 128 + rank
    blk = col_off2[sgrp] + cls_off2[sgrp, cls_s]
    slotpos = blk * 128 + i_call
    idxrow = i_call % 16
    idxcol = 8 * blk + i_call // 16
    gl = slotpos // 128 - col_off2[sgrp]
    gj = gl % 4
    gcol = (sgrp_off[sgrp] + gl // 4) * 128 + slotpos % 128

    out = []
    nslots = totcols * 128
    xs_s = x[src_s].astype(np.float32)
    dl_s = (dloc_s - tile_s * 128).astype(np.float32)
    bounds = np.searchsorted(core_s, np.arange(NCORES + 1))
    for k in range(NCORES):
        m = slice(bounds[k], bounds[k + 1])
        sp = slotpos[m]
        sloc = np.full(nslots, -1.0, np.float32)
        sloc[sp] = dl_s[m]
        p3g = np.zeros((12, (totgrp + maxgrp2) * 128), np.float32)
        p3g[3 * gj[m] + 0, gcol[m]] = xs_s[m]
        p3g[3 * gj[m] + 1, gcol[m]] = ea_s[m]
        p3g[3 * gj[m] + 2, gcol[m]] = 1.0
        idx16 = np.zeros((16, 8 * totcols), np.int16)
        idx16[idxrow[m], idxcol[m]] = q16[m]
        out.append({
            "sloc": np.ascontiguousarray(sloc.reshape(totcols, 128).T).astype(bf),
            "p3g": p3g.astype(bf),
            "idx16": np.tile(idx16, (8, 1)),
        })
    return out


# ---------------------------------------------------------------- the launch


def _build(chunks, chcls, chq, cls_off2, col_off2, ngrp2, sgrp_off,
           totcols, totgrp, eps1p1):
    NS = NT // 2
    chunks2 = chunks[0::2] + chunks[1::2]
    maxch2 = max(int(chunks2.max()), 1)
    maxgrp2 = max(int(ngrp2.max()), 1)
    nc = bass.Bass(target_bir_lowering=False, debug=False, num_swdge_queues=4)
    p3g = nc.declare_dram_parameter("p3g", [12, (totgrp + maxgrp2) * 128], BF16, isOutput=False)
    dstf = nc.declare_dram_parameter("dstf", [128, totcols], BF16, isOutput=False)
    idx16 = nc.declare_dram_parameter("idx16", [128, 8 * totcols], I16, isOutput=False)
    xin = nc.declare_dram_parameter("x", [1, NPAD], F32, isOutput=False)
    encw = nc.declare_dram_parameter("encw", [1, H], F32, isOutput=False)
    w3blk = nc.declare_dram_parameter("w3blk", [12, 512], BF16, isOutput=False)
    vblk12 = nc.declare_dram_parameter("vblk12", [12, 512], BF16, isOutput=False)
    iotab = nc.declare_dram_parameter("iotab", [128, 128], BF16, isOutput=False)
    w1l0 = nc.declare_dram_parameter("w1l0", [H, H], BF16, isOutput=False)
    w2l0 = nc.declare_dram_parameter("w2l0", [H, H], BF16, isOutput=False)
    w1l1 = nc.declare_dram_parameter("w1l1", [H, H], BF16, isOutput=False)
    w2l1 = nc.declare_dram_parameter("w2l1", [H, H], BF16, isOutput=False)
    vb1l0 = nc.declare_dram_parameter("vb1l0", [H, 1], F32, isOutput=False)
    vb1l1 = nc.declare_dram_parameter("vb1l1", [H, 1], F32, isOutput=False)
    vheps = nc.declare_dram_parameter("vheps", [H, 1], F32, isOutput=False)
    vbnb0 = nc.declare_dram_parameter("vbnb0", [H, 1], F32, isOutput=False)
    repk = nc.declare_dram_parameter("repk", [128, 576 + NT], F32, isOutput=False)
    cw1 = nc.declare_dram_parameter("cw1", [H, G], F32, isOutput=False)
    vcb1 = nc.declare_dram_parameter("vcb1", [G, 1], F32, isOutput=False)
    cw2 = nc.declare_dram_parameter("cw2", [G, 1], F32, isOutput=False)
    cb2 = nc.declare_dram_parameter("cb2", [1, 1], F32, isOutput=False)
    out_o = nc.declare_dram_parameter("out", [1, G], F32, isOutput=True)
    dbg = os.environ.get("KERNEL_DEBUG_OUT") == "1"
    if dbg:
        h1dbg = nc.declare_dram_parameter("h1dbg", [NPAD, H], BF16, isOutput=True)
        pooldbg = nc.declare_dram_parameter("pooldbg", [H, G], F32, isOutput=True)
        z2dbg = nc.declare_dram_parameter("z2dbg", [H, NPAD], F32, isOutput=True)

    with tile.TileContext(nc) as tc:
        with (
            tc.tile_pool(name="const", bufs=1) as cp,
            tc.tile_pool(name="sb", bufs=3) as sb,
            tc.tile_pool(name="big", bufs=1) as bigp,
            tc.tile_pool(name="psE", bufs=2, space="PSUM") as psE,
            tc.tile_pool(name="psA", bufs=2, space="PSUM") as psA,
            tc.tile_pool(name="psN", bufs=2, space="PSUM") as psN,
            tc.tile_pool(name="psP", bufs=1, space="PSUM") as psP,
            tc.tile_pool(name="dram", bufs=1, space="DRAM") as dram,
        ):
            def ld(param, shape, dtype=F32):
                t = cp.tile(shape, dtype, tag=f"c_{param.name}")
                nc.sync.dma_start(t[:], param[:, :])
                return t

            w3blk_t = ld(w3blk, [12, 512], BF16)
            vblk12_t = ld(vblk12, [12, 512], BF16)
            w1l0_t = ld(w1l0, [H, H], BF16)
            w2l0_t = ld(w2l0, [H, H], BF16)
            w1l1_t = ld(w1l1, [H, H], BF16)
            w2l1_t = ld(w2l1, [H, H], BF16)
            vb1l0_t = ld(vb1l0, [H, 1])
            vb1l1_t = ld(vb1l1, [H, 1])
            vheps_t = ld(vheps, [H, 1])
            vbnb0_t = ld(vbnb0, [H, 1])
            repk_t = ld(repk, [128, 576 + NT])
            cw1_t = ld(cw1, [H, G])
            vcb1_t = ld(vcb1, [G, 1])
            cw2_t = ld(cw2, [G, 1])
            cb2_t = ld(cb2, [1, 1])
            encw_t = ld(encw, [1, H])
            dstall_t = ld(dstf, [128, totcols], BF16)
            iotab_t = ld(iotab, [128, 128], BF16)
            nc.gpsimd.load_library(library_config.mlp)

            # ---- phase 1: hepsT[f, n] = (1+eps0) * (encw[f]*x[n] + encb[f])
            # (x and vheps pre-scaled by (1+eps0) on host)
            hepsT = bigp.tile([H, NPAD], BF16)
            for c0n in range(0, NPAD, 512):
                w = min(512, NPAD - c0n)
                x_t = sb.tile([1, 512], F32, tag="xchunk")
                nc.sync.dma_start(x_t[:, :w], xin[:, c0n : c0n + w])
                ps = psE.tile([H, 512], F32, space="PSUM", tag="mmE")
                nc.tensor.matmul(
                    out=ps[:, :w], lhsT=encw_t[:], rhs=x_t[:, :w],
                    start=True, stop=True,
                )
                nc.vector.tensor_tensor(
                    out=hepsT[:, c0n : c0n + w], in0=ps[:, :w],
                    in1=vheps_t[:].to_broadcast([H, w]), op=ADD,
                )

            # ---- phase 2: layer-1 message expansion + scatter (per supertile)
            zT = bigp.tile([H, NPAD], BF16)

            def scatter_tile(s, i, msg_t, oh_t, agg_pool):
                t = 2 * s + i
                agg = agg_pool.tile([H, 128], F32, space="PSUM", tag="agg")
                ntot = int(chunks[t])
                done = 0
                for r in range(4):
                    boff = int(cls_off2[s][r]) + (int(chcls[t - 1][r]) if i else 0)
                    for cc in range(int(chcls[t][r])):
                        col = boff + cc
                        done += 1
                        nc.tensor.matmul(
                            out=agg[:],
                            lhsT=msg_t[:, col, :],
                            rhs=oh_t[:, col, :],
                            start=(done == 1),
                            stop=(done == ntot),
                        )
                nc.vector.tensor_tensor(
                    out=zT[:, t * 128 : (t + 1) * 128],
                    in0=agg[:],
                    in1=hepsT[:, t * 128 : (t + 1) * 128],
                    op=ADD,
                )

            # ---- phase 3 (emitted interleaved): layer-1 GIN MLP; h1
            # feature-major (scaled by 1+eps1, overwrites hepsT) and
            # node-major bf16 rows (+ eenc bias) for the gather tables
            h1locq = []
            for q in range(4):
                h1q = dram.tile([int(QS[q]), H], BF16, name=f"h1locq{q}",
                                tag=f"h1locq{q}")
                h1locq.append(h1q)

            def mlp1_chunk(c0n):
                w = min(512, NPAD - c0n)
                ps1 = psE.tile([H, 512], F32, space="PSUM", tag="mmE",
                               name="ps1")
                nc.tensor.matmul(
                    out=ps1[:, :w], lhsT=w1l0_t[:], rhs=zT[:, c0n : c0n + w],
                    start=True, stop=True,
                )
                y1 = sb.tile([H, 512], BF16, tag="y1", name="y1")
                nc.scalar.activation(
                    out=y1[:, :w], in_=ps1[:, :w], func=RELU, bias=vb1l0_t[:]
                )
                ps2 = psE.tile([H, 512], F32, space="PSUM", tag="mmE",
                               name="ps2")
                nc.tensor.matmul(
                    out=ps2[:, :w], lhsT=w2l0_t[:], rhs=y1[:, :w],
                    start=True, stop=True,
                )
                nc.scalar.activation(
                    out=hepsT[:, c0n : c0n + w], in_=ps2[:, :w], func=RELU,
                    bias=vbnb0_t[:],
                )
                nc.vector.tensor_scalar(
                    out=hepsT[:, c0n : c0n + w], in0=hepsT[:, c0n : c0n + w],
                    scalar1=eps1p1, scalar2=None, op0=MULT,
                )
                for sub in range(w // 128):
                    n0 = c0n + sub * 128
                    ps2n = psN.tile([128, H], F32, space="PSUM", tag="mmN",
                                    name="ps2n")
                    nc.tensor.matmul(
                        out=ps2n[:],
                        lhsT=y1[:, sub * 128 : (sub + 1) * 128],
                        rhs=w2l0_t[:],
                        start=True, stop=True,
                    )
                    h1n = sb.tile([128, H], F32, tag="h1n", name="h1n")
                    nc.vector.tensor_tensor(
                        out=h1n[:], in0=ps2n[:], in1=repk_t[:, 0:128], op=ADD
                    )
                    nc.vector.tensor_scalar(
                        out=h1n[:], in0=h1n[:], scalar1=0.0, scalar2=None, op0=MAX
                    )
                    h1nb = sb.tile([128, H], BF16, tag="h1nb", name="h1nb")
                    nc.vector.tensor_tensor(
                        out=h1nb[:], in0=h1n[:], in1=repk_t[:, 256:384], op=ADD
                    )
                    q = int(np.searchsorted(QB[1:], n0, side="right"))
                    nc.sync.dma_start(
                        h1locq[q][n0 - int(QB[q]) : n0 - int(QB[q]) + 128, :],
                        h1nb[:],
                    )
                    if dbg:
                        nc.sync.dma_start(h1dbg[n0 : n0 + 128, :], h1nb[:])

            for s in range(NS):
                scol = int(chunks2[s])
                c0 = int(col_off2[s])
                g0 = int(sgrp_off[s])
                ngrp = int(ngrp2[s])
                p3_t = sb.tile([12, maxgrp2 * 128], BF16, tag="p3")
                nc.sync.dma_start(
                    p3_t[:, :], p3g[:, g0 * 128 : (g0 + maxgrp2) * 128]
                )
                msg_t = sb.tile([128, maxch2, H], BF16, tag="msg")
                for gi in range(ngrp):
                    ps = psE.tile([128, 512], F32, space="PSUM", tag="mmE")
                    nsub = min(4, scol - gi * 4)
                    nc.tensor.matmul(
                        out=ps[:],
                        lhsT=p3_t[:, gi * 128 : (gi + 1) * 128],
                        rhs=w3blk_t[:],
                        start=True, stop=True,
                    )
                    nc.scalar.activation(
                        out=msg_t[:, gi * 4 : gi * 4 + nsub, :],
                        in_=ps[:, : nsub * 128],
                        func=RELU,
                    )
                oh_t = sb.tile([128, maxch2, 128], BF16, tag="oh")
                nc.vector.tensor_tensor(
                    out=oh_t[:, :scol, :],
                    in0=dstall_t[:, c0 : c0 + scol].unsqueeze(2).to_broadcast([128, scol, 128]),
                    in1=iotab_t[:].unsqueeze(1).to_broadcast([128, scol, 128]),
                    op=ISEQ,
                )
                scatter_tile(s, 0, msg_t, oh_t, psA)
                scatter_tile(s, 1, msg_t, oh_t, psA)
                if s % 2 == 1:
                    mlp1_chunk((s // 2) * 512)
            mlp1_chunk(12288)

            # ---- phase 4: AllGather the h1 table as 4 quarter tables,
            # issued as each quarter's rows complete (overlaps phase 3)
            tq = []
            for q in range(4):
                qb, qs = int(QB[q]), int(QS[q])
                tq_q = dram.tile(
                    [NCORES * qs, H], BF16, addr_space="Shared",
                    name=f"tq{q}", tag=f"tq{q}",
                )
                nc.gpsimd.collective_compute(
                    "AllGather", mybir.AluOpType.bypass,
                    replica_groups=[list(range(NCORES))],
                    ins=[h1locq[q].opt()], outs=[tq_q.opt()],
                )
                tq.append(tq_q)

            # ---- phase 6 (emitted interleaved with phase 5):
            # layer-2 GIN MLP (node-major) + pooling
            pool_ps = psP.tile([H, G], F32, space="PSUM", tag="pool")

            def mlp2_chunk(c0n):
                w = min(512, NPAD - c0n)
                ps1 = psE.tile([H, 512], F32, space="PSUM", tag="mmE",
                               name="ps1b")
                nc.tensor.matmul(
                    out=ps1[:, :w], lhsT=w1l1_t[:], rhs=zT[:, c0n : c0n + w],
                    start=True, stop=True,
                )
                y1 = sb.tile([H, 512], BF16, tag="y1", name="y1b")
                nc.scalar.activation(
                    out=y1[:, :w], in_=ps1[:, :w], func=RELU, bias=vb1l1_t[:]
                )
                for sub in range(w // 128):
                    t = (c0n + sub * 128) // 128
                    ps2n = psN.tile([128, H], F32, space="PSUM", tag="mmN",
                                    name="ps2nb")
                    nc.tensor.matmul(
                        out=ps2n[:],
                        lhsT=y1[:, sub * 128 : (sub + 1) * 128],
                        rhs=w2l1_t[:],
                        start=True, stop=True,
                    )
                    h2n = sb.tile([128, H], F32, tag="h2n", name="h2n")
                    nc.vector.tensor_tensor(
                        out=h2n[:], in0=ps2n[:], in1=repk_t[:, 128:256], op=ADD
                    )
                    nc.vector.tensor_scalar(
                        out=h2n[:], in0=h2n[:], scalar1=0.0, scalar2=None, op0=MAX
                    )
                    gb_oh = sb.tile([128, G], F32, tag="gboh", name="gb_oh")
                    nc.vector.tensor_tensor(
                        out=gb_oh[:],
                        in0=repk_t[:, 576 + t : 577 + t].to_broadcast([128, G]),
                        in1=repk_t[:, 384:448],
                        op=ISEQ,
                    )
                    nc.tensor.matmul(
                        out=pool_ps[:],
                        lhsT=h2n[:],
                        rhs=gb_oh[:],
                        start=(t == 0),
                        stop=(t == NT - 1),
                    )

            # ---- phase 5: layer-2 gather + scatter (per supertile)
            nidx_regs = {}
            for s in range(NS):
                for r in range(4):
                    cq = int(chq[s][r])
                    if cq and cq not in nidx_regs:
                        nidx_regs[cq] = nc.gpsimd.to_reg(cq * 128)
            for s in range(NS):
                scol = int(chunks2[s])
                c0 = int(col_off2[s])
                g0 = int(sgrp_off[s])
                ngrp = int(ngrp2[s])
                idx_t = sb.tile([128, 8 * maxch2], I16, tag="idx16")
                nc.sync.dma_start(
                    idx_t[:, : 8 * scol], idx16[:, 8 * c0 : 8 * (c0 + scol)]
                )
                msg_t = sb.tile([128, maxch2, H], BF16, tag="msg")
                for r in range(4):
                    cq = int(chq[s][r])
                    if cq == 0:
                        continue
                    coff = int(cls_off2[s][r])
                    nc.gpsimd.dma_gather(
                        out_ap=msg_t[:, coff : coff + cq, :],
                        in_ap=tq[r][:, :],
                        idxs_ap=idx_t[:, 8 * coff : 8 * (coff + cq)],
                        num_idxs=cq * 128,
                        num_idxs_reg=nidx_regs[cq],
                        elem_size=H,
                        single_packet=False,
                        queue_num=r,
                    )
                p3_t = sb.tile([12, maxgrp2 * 128], BF16, tag="p3")
                nc.sync.dma_start(
                    p3_t[:, :], p3g[:, g0 * 128 : (g0 + maxgrp2) * 128]
                )
                eav_t = sb.tile([128, maxch2, H], BF16, tag="eav")
                for gi in range(ngrp):
                    ps = psE.tile([128, 512], F32, space="PSUM", tag="mmE")
                    nsub = min(4, scol - gi * 4)
                    nc.tensor.matmul(
                        out=ps[:],
                        lhsT=p3_t[:, gi * 128 : (gi + 1) * 128],
                        rhs=vblk12_t[:],
                        start=True, stop=True,
                    )
                    nc.scalar.activation(
                        out=eav_t[:, gi * 4 : gi * 4 + nsub, :],
                        in_=ps[:, : nsub * 128],
                        func=COPY,
                    )
                nc.vector.tensor_tensor(
                    out=msg_t[:, :scol, :], in0=msg_t[:, :scol, :],
                    in1=eav_t[:, :scol, :], op=ADD,
                )
                nc.vector.tensor_scalar(
                    out=msg_t[:, :scol, :], in0=msg_t[:, :scol, :],
                    scalar1=0.0, scalar2=None, op0=MAX,
                )
                oh_t = sb.tile([128, maxch2, 128], BF16, tag="oh")
                nc.vector.tensor_tensor(
                    out=oh_t[:, :scol, :],
                    in0=dstall_t[:, c0 : c0 + scol].unsqueeze(2).to_broadcast([128, scol, 128]),
                    in1=iotab_t[:].unsqueeze(1).to_broadcast([128, scol, 128]),
                    op=ISEQ,
                )
                scatter_tile(s, 0, msg_t, oh_t, psA)
                scatter_tile(s, 1, msg_t, oh_t, psA)
                if s % 2 == 1:
                    mlp2_chunk((s // 2) * 512)
            mlp2_chunk(12288)

            # ---- phase 7: AllReduce pool partials + classifier
            poolbuf = sb.tile([H, G], F32, tag="poolo")
            nc.vector.tensor_copy(out=poolbuf[:], in_=pool_ps[:])
            poolin = dram.tile([H, G], F32)
            nc.sync.dma_start(poolin[:], poolbuf[:])
            if dbg:
                nc.sync.dma_start(pooldbg[:, :], poolbuf[:])
                nc.sync.dma_start(z2dbg[:, :], zT[:])
            poolsum = dram.tile([H, G], F32, addr_space="Shared")
            nc.gpsimd.collective_compute(
                "AllReduce", ADD,
                replica_groups=[list(range(NCORES))],
                ins=[poolin.opt()], outs=[poolsum.opt()],
            )
            pooled = sb.tile([H, G], F32, tag="pooled")
            nc.sync.dma_start(pooled[:], poolsum[:])
            nc.vector.tensor_tensor(
                out=pooled[:], in0=pooled[:], in1=repk_t[:, 512:576], op=MULT
            )
            ps_c1 = psN.tile([128, H], F32, space="PSUM", tag="mmN")
            nc.tensor.matmul(
                out=ps_c1[:G, :G], lhsT=cw1_t[:], rhs=pooled[:], start=True, stop=True
            )
            hc = sb.tile([G, G], F32, tag="hc")
            nc.scalar.activation(
                out=hc[:], in_=ps_c1[:G, :G], func=RELU, bias=vcb1_t[:]
            )
            ps_c2 = psN.tile([128, H], F32, space="PSUM", tag="mmN")
            nc.tensor.matmul(
                out=ps_c2[:1, :G], lhsT=cw2_t[:], rhs=hc[:], start=True, stop=True
            )
            ob = sb.tile([1, G], F32, tag="ob")
            nc.scalar.activation(out=ob[:], in_=ps_c2[:1, :G], func=SIG, bias=cb2_t[:])
            nc.sync.dma_start(out_o[:, :], ob[:])
    return nc


# ---------------------------------------------------------------- driver


def kernel(x, edge_index, edge_attr, batch,
           enc_w, enc_b, eenc_w, eenc_b,
           eps, w1, b1, w2, b2, gamma, beta,
           cw1, cb1, cw2, cb2):
    _install_fixups()
    x = np.asarray(x, np.float32).reshape(-1)
    src = np.asarray(edge_index[0], np.int64)
    dst = np.asarray(edge_index[1], np.int64)
    ea = np.asarray(edge_attr, np.float32).reshape(-1)
    batch = np.asarray(batch, np.int64)
    inv_std = np.float32(1.0 / np.sqrt(1.0 + BN_EPS))
    cores = list(range(NCORES))

    (order, core_s, tile_s, cls_s, dloc_s, starts, chunks, chcls, chq,
     cls_off2, col_off2, ngrp2, sgrp_off, totcols, totgrp) = _edge_meta(src, dst)

    u = np.asarray(enc_w, np.float32).reshape(H)
    v = np.asarray(eenc_w, np.float32).reshape(H)
    b0 = np.asarray(enc_b, np.float32).reshape(H)
    be = np.asarray(eenc_b, np.float32).reshape(H)
    iota_np = np.tile(np.arange(128, dtype=np.float32)[None, :], (128, 1))
    eps_np = np.asarray(eps, np.float32)

    def bn_fold(l):
        g = np.asarray(gamma, np.float32)[l]
        s = g * inv_std
        bvec = np.asarray(beta, np.float32)[l] + s * np.asarray(b2, np.float32)[l]
        return s, bvec

    bns0, bnb0 = bn_fold(0)
    bns1, bnb1 = bn_fold(1)
    w1_np = np.asarray(w1, np.float32)
    w2_np = np.asarray(w2, np.float32)
    b1_np = np.asarray(b1, np.float32)
    w2l0 = w2_np[0] * bns0[None, :]
    w2l1 = w2_np[1] * bns1[None, :]

    cnt = np.bincount(batch, minlength=G).astype(np.float32)
    invg = (1.0 / np.maximum(cnt, 1.0)).astype(np.float32)

    maxgrp2 = max(int(ngrp2.max()), 1)

    nc = _build(chunks, chcls, chq, cls_off2, col_off2, ngrp2, sgrp_off,
                totcols, totgrp, float(1.0 + eps_np[1]))
    slot_pc = _edge_fill(
        src, ea, x, order, core_s, tile_s, cls_s, dloc_s, starts,
        chcls, cls_off2, col_off2, sgrp_off, totcols, totgrp, maxgrp2,
    )

    # block-diag expansion weights: rows 3j+r carry w3 row r in block j
    w3blk_np = np.zeros((12, 512), np.float32)
    vblk12_np = np.zeros((12, 512), np.float32)
    w3f = np.stack([u, v, b0 + be])
    for j in range(4):
        for r in range(3):
            w3blk_np[3 * j + r, 128 * j : 128 * (j + 1)] = w3f[r]
        vblk12_np[3 * j + 1, 128 * j : 128 * (j + 1)] = v


    in_maps = []
    for k in cores:
        pc = slot_pc[k]
        xloc = np.zeros(NPAD, np.float32)
        xloc[:NLOC] = x[k * NLOC : (k + 1) * NLOC]
        gloc = np.full(NPAD, -1.0, np.float32)
        gloc[:NLOC] = batch[k * NLOC : (k + 1) * NLOC].astype(np.float32)
        in_maps.append({
            "p3g": pc["p3g"],
            "dstf": pc["sloc"],
            "idx16": pc["idx16"],
            "x": (xloc * (1.0 + eps_np[0])).reshape(1, NPAD),
            "encw": u.reshape(1, H),
            "w3blk": w3blk_np.astype(bf),
            "vblk12": vblk12_np.astype(bf),
            "iotab": iota_np.astype(bf),
            "w1l0": w1_np[0].astype(bf), "w2l0": w2l0.astype(bf),
            "w1l1": w1_np[1].astype(bf), "w2l1": w2l1.astype(bf),
            "vb1l0": b1_np[0].reshape(H, 1),
            "vb1l1": b1_np[1].reshape(H, 1),
            "vheps": ((1.0 + eps_np[0]) * b0).reshape(H, 1),
            "vbnb0": bnb0.reshape(H, 1),
            "repk": np.concatenate([
                np.tile(bnb0[None, :], (128, 1)),
                np.tile(bnb1[None, :], (128, 1)),
                np.tile(be[None, :], (128, 1)),
                iota_np,
                np.tile(invg[None, :], (128, 1)),
                np.ascontiguousarray(gloc.reshape(NT, 128).T),
            ], axis=1).astype(np.float32),
            "cw1": np.asarray(cw1, np.float32),
            "vcb1": np.asarray(cb1, np.float32).reshape(G, 1),
            "cw2": np.asarray(cw2, np.float32),
            "cb2": np.asarray(cb2, np.float32).reshape(1, 1),
        })

    library_overlay.lower_extended_insts(nc)
    res = run_bass_kernel_spmd(nc, in_maps, cores)
    kernel.last_results = res
    return res.results[0]["out"].reshape(G).astype(np.float32)



# revision 55
# speedup vs baseline: 1.2509x; 1.2509x over previous
"""GINEConv (2-layer, N=100k, E=1.6M, H=128, G=64) on 8 Trainium2 cores.

Single SPMD launch. Nodes + incident (dst) edges partitioned across 8
cores; edges dst-sorted and tiled into 128-dst windows; per-window
aggregation is one-hot scatter-matmuls accumulating in PSUM. Layer-1
messages are expanded on-device from (x_src, edge_attr, 1) streams via a
K=3 matmul + ReLU. h1 rows are written node-major (bf16) to DRAM, then
AllGather'd across the 8 cores; layer-2 per-edge h1[src] rows come from
bulk dma_gather calls (4 per tile: slots grouped by src_row mod 4 so
int16 class-local indices address a 1024B-stride view of the table),
then a DVE add of the edge-encoder term + relu. Pooling is a per-core
one-hot matmul + AllReduce; the classifier runs redundantly on every
core.
"""

import json
import os

try:
    import orjson as _fastjson
except ImportError:
    _fastjson = None

import ml_dtypes
import numpy as np

import concourse.bass as bass
import concourse.bass2jax as _b2j
import concourse.bass_utils as _bu
import concourse.tile as tile
from concourse import library_config, library_overlay, mybir
from concourse.bass_utils import run_bass_kernel_spmd

F32 = mybir.dt.float32
BF16 = mybir.dt.bfloat16
I32 = mybir.dt.int32
I16 = mybir.dt.int16
RELU = mybir.ActivationFunctionType.Relu
SIG = mybir.ActivationFunctionType.Sigmoid
COPY = mybir.ActivationFunctionType.Copy
ADD = mybir.AluOpType.add
MULT = mybir.AluOpType.mult
MAX = mybir.AluOpType.max
ISEQ = mybir.AluOpType.is_equal

N, E, H, G = 100000, 1600000, 128, 64
NCORES = 8
NLOC = N // NCORES            # 12500 nodes per core
NPAD = 12544                  # padded to 98*128
NT = NPAD // 128              # 98 node tiles per core
BN_EPS = 1e-5
bf = ml_dtypes.bfloat16

# ---------------------------------------------------------------- fixups


_WS_CTR = [0]


def _split_multiwait_bir(bir_json):
    data = bir_json.decode() if isinstance(bir_json, (bytes, bytearray)) else bir_json
    bir = _fastjson.loads(data) if _fastjson else json.loads(data)
    changed = False
    for f in bir.get("functions", []):
        for b in f.get("blocks", []):
            out = []
            for inst in b.get("instructions", []):
                si = inst.get("sync_info") or {}
                waits = si.get("on_wait") or []
                if len(waits) > 1:
                    changed = True
                    for w in waits[:-1]:
                        _WS_CTR[0] += 1
                        nop = {
                            "name": f"I-wsplit-{_WS_CTR[0]}",
                            "opcode": "NoOp",
                            "engine": inst["engine"],
                            "ins": [],
                            "outs": [],
                            "sync_info": {"on_update": [], "on_wait": [w]},
                        }
                        if "debug" in inst:
                            nop["debug"] = inst["debug"]
                        out.append(nop)
                    si["on_wait"] = [waits[-1]]
                out.append(inst)
            b["instructions"] = out
    if not changed:
        return bir_json
    return _fastjson.dumps(bir) if _fastjson else json.dumps(bir).encode()


_ORIG_COMPILE = _bu.compile_bir_kernel


def _patched_compile(bir_json, *args, **kwargs):
    return _ORIG_COMPILE(_split_multiwait_bir(bir_json), *args, **kwargs)


def _install_fixups():
    _bu.compile_bir_kernel = _patched_compile
    _b2j.compile_bir_kernel = _patched_compile


# ---------------------------------------------------------------- host prep


QB = np.array([0, 3200, 6400, 9472], np.int64)
QS = np.array([3200, 3200, 3072, 3072], np.int64)


def _edge_meta(src, dst):
    """Cheap prefix: dst-sort with src-quarter class grouping per (core, tile).

    Classes are src-node quarters (by local row range) so the layer-2 gather
    uses int16 table-local indices into one of four AllGather'd quarter
    tables. Columns are laid out per SUPERTILE (pair of dst tiles),
    class-major, so each (supertile, class) is one contiguous gather call.
    """
    core = dst // NLOC
    dloc = dst - core * NLOC
    tile_id = dloc // 128
    src_core = src // NLOC
    src_loc = src - src_core * NLOC
    cls = np.searchsorted(QB[1:], src_loc, side="right")
    order = np.lexsort((dloc, cls, tile_id, core))
    core_s, tile_s, cls_s, dloc_s = (
        core[order], tile_id[order], cls[order], dloc[order],
    )

    counts = np.zeros((NCORES, NT, 4), np.int64)
    np.add.at(counts, (core_s, tile_s, cls_s), 1)
    chcls = (counts.max(axis=0) + 127) // 128          # [NT, 4] chunk cols
    chunks = chcls.sum(axis=1)                         # [NT]
    chunks2 = chunks[0::2] + chunks[1::2]              # [NS] supertile cols
    col_off2 = np.concatenate([[0], chunks2.cumsum()])[:-1]
    totcols = int(chunks2.sum())
    chq = chcls[0::2] + chcls[1::2]                    # [NS, 4]
    cls_off2 = np.zeros((NT // 2, 4), np.int64)
    cls_off2[:, 1:] = chq.cumsum(axis=1)[:, :-1]
    ngrp2 = (chunks2 + 3) // 4
    sgrp_off = np.concatenate([[0], ngrp2.cumsum()])[:-1]
    totgrp = int(ngrp2.sum())

    starts = np.zeros(NCORES * NT * 4, np.int64)
    flat = counts.reshape(-1).cumsum()
    starts[1:] = flat[:-1]
    return (order, core_s, tile_s, cls_s, dloc_s, starts, chunks, chcls,
            chq, cls_off2, col_off2, ngrp2, sgrp_off, totcols, totgrp)


def _edge_fill(src, ea, x, order, core_s, tile_s, cls_s, dloc_s, starts,
               chcls, cls_off2, col_off2, sgrp_off, totcols, totgrp, maxgrp2):
    """Heavy per-core slot streams; run in a worker thread.

    Supertile s covers dst tiles (2s, 2s+1); its columns are class-major:
    [cls0: t-even chunks, t-odd chunks][cls1: ...]. Slot i of a
    (supertile, class) gather call -> partition i%128, chunk-col i//128
    within the class block.
    Returns per-core dict:
      sloc  bf16 [128, totcols] dst index within its own tile (-1 pads)
      p3g   bf16 [12, (totgrp+maxgrp2)*128] grouped (x, ea, 1) stream per
            supertile (groups of 4 consecutive supertile columns)
      idx16 i16  [128, 8*totcols] quarter-table gather indices packed
            i%16 / i//16, replicated 8x across partitions
    """
    src_s = src[order]
    ea_s = ea[order]
    src_core = src_s // NLOC
    src_loc = src_s - src_core * NLOC
    q16 = (src_core * QS[cls_s] + (src_loc - QB[cls_s])).astype(np.int16)

    run_id = (core_s * NT + tile_s) * 4 + cls_s
    rank = np.arange(len(src_s), dtype=np.int64) - starts[run_id]
    sgrp = tile_s // 2
    par = tile_s % 2
    i_call = par * chcls[tile_s - par, cls_s] * 128 + rank
    blk = col_off2[sgrp] + cls_off2[sgrp, cls_s]
    slotpos = blk * 128 + i_call
    idxrow = i_call % 16
    idxcol = 8 * blk + i_call // 16
    gl = slotpos // 128 - col_off2[sgrp]
    gj = gl % 4
    gcol = (sgrp_off[sgrp] + gl // 4) * 128 + slotpos % 128

    out = []
    nslots = totcols * 128
    xs_s = x[src_s].astype(np.float32)
    dl_s = (dloc_s - tile_s * 128).astype(np.float32)
    bounds = np.searchsorted(core_s, np.arange(NCORES + 1))
    for k in range(NCORES):
        m = slice(bounds[k], bounds[k + 1])
        sp = slotpos[m]
        sloc = np.full(nslots, -1.0, np.float32)
        sloc[sp] = dl_s[m]
        p3g = np.zeros((12, (totgrp + maxgrp2) * 128), np.float32)
        p3g[3 * gj[m] + 0, gcol[m]] = xs_s[m]
        p3g[3 * gj[m] + 1, gcol[m]] = ea_s[m]
        p3g[3 * gj[m] + 2, gcol[m]] = 1.0
        idx16 = np.zeros((16, 8 * totcols), np.int16)
        idx16[idxrow[m], idxcol[m]] = q16[m]
        out.append({
            "sloc": np.ascontiguousarray(sloc.reshape(totcols, 128).T).astype(bf),
            "p3g": p3g.astype(bf),
            "idx16": np.tile(idx16, (8, 1)),
        })
    return out


# ---------------------------------------------------------------- the launch


def _build(chunks, chcls, chq, cls_off2, col_off2, ngrp2, sgrp_off,
           totcols, totgrp, eps1p1):
    NS = NT // 2
    chunks2 = chunks[0::2] + chunks[1::2]
    maxch2 = max(int(chunks2.max()), 1)
    maxgrp2 = max(int(ngrp2.max()), 1)
    nc = bass.Bass(target_bir_lowering=False, debug=False, num_swdge_queues=4)
    p3g = nc.declare_dram_parameter("p3g", [12, (totgrp + maxgrp2) * 128], BF16, isOutput=False)
    dstf = nc.declare_dram_parameter("dstf", [128, totcols], BF16, isOutput=False)
    idx16 = nc.declare_dram_parameter("idx16", [128, 8 * totcols], I16, isOutput=False)
    xin = nc.declare_dram_parameter("x", [1, NPAD], F32, isOutput=False)
    encw = nc.declare_dram_parameter("encw", [1, H], F32, isOutput=False)
    w3blk = nc.declare_dram_parameter("w3blk", [12, 512], BF16, isOutput=False)
    vblk12 = nc.declare_dram_parameter("vblk12", [12, 512], BF16, isOutput=False)
    iotab = nc.declare_dram_parameter("iotab", [128, 128], BF16, isOutput=False)
    w1l0 = nc.declare_dram_parameter("w1l0", [H, H], BF16, isOutput=False)
    w2l0 = nc.declare_dram_parameter("w2l0", [H, H], BF16, isOutput=False)
    w1l1 = nc.declare_dram_parameter("w1l1", [H, H], BF16, isOutput=False)
    w2l1 = nc.declare_dram_parameter("w2l1", [H, H], BF16, isOutput=False)
    vb1l0 = nc.declare_dram_parameter("vb1l0", [H, 1], F32, isOutput=False)
    vb1l1 = nc.declare_dram_parameter("vb1l1", [H, 1], F32, isOutput=False)
    vheps = nc.declare_dram_parameter("vheps", [H, 1], F32, isOutput=False)
    vbnb0 = nc.declare_dram_parameter("vbnb0", [H, 1], F32, isOutput=False)
    repk = nc.declare_dram_parameter("repk", [128, 576 + NT], F32, isOutput=False)
    cw1 = nc.declare_dram_parameter("cw1", [H, G], F32, isOutput=False)
    vcb1 = nc.declare_dram_parameter("vcb1", [G, 1], F32, isOutput=False)
    cw2 = nc.declare_dram_parameter("cw2", [G, 1], F32, isOutput=False)
    cb2 = nc.declare_dram_parameter("cb2", [1, 1], F32, isOutput=False)
    out_o = nc.declare_dram_parameter("out", [1, G], F32, isOutput=True)
    dbg = os.environ.get("KERNEL_DEBUG_OUT") == "1"
    if dbg:
        h1dbg = nc.declare_dram_parameter("h1dbg", [NPAD, H], BF16, isOutput=True)
        pooldbg = nc.declare_dram_parameter("pooldbg", [H, G], F32, isOutput=True)
        z2dbg = nc.declare_dram_parameter("z2dbg", [H, NPAD], F32, isOutput=True)

    with tile.TileContext(nc) as tc:
        with (
            tc.tile_pool(name="const", bufs=1) as cp,
            tc.tile_pool(name="sb", bufs=3) as sb,
            tc.tile_pool(name="big", bufs=1) as bigp,
            tc.tile_pool(name="psE", bufs=2, space="PSUM") as psE,
            tc.tile_pool(name="psA", bufs=2, space="PSUM") as psA,
            tc.tile_pool(name="psN", bufs=2, space="PSUM") as psN,
            tc.tile_pool(name="psP", bufs=1, space="PSUM") as psP,
            tc.tile_pool(name="dram", bufs=1, space="DRAM") as dram,
        ):
            def ld(param, shape, dtype=F32):
                t = cp.tile(shape, dtype, tag=f"c_{param.name}")
                nc.sync.dma_start(t[:], param[:, :])
                return t

            w3blk_t = ld(w3blk, [12, 512], BF16)
            vblk12_t = ld(vblk12, [12, 512], BF16)
            w1l0_t = ld(w1l0, [H, H], BF16)
            w2l0_t = ld(w2l0, [H, H], BF16)
            w1l1_t = ld(w1l1, [H, H], BF16)
            w2l1_t = ld(w2l1, [H, H], BF16)
            vb1l0_t = ld(vb1l0, [H, 1])
            vb1l1_t = ld(vb1l1, [H, 1])
            vheps_t = ld(vheps, [H, 1])
            vbnb0_t = ld(vbnb0, [H, 1])
            repk_t = ld(repk, [128, 576 + NT])
            cw1_t = ld(cw1, [H, G])
            vcb1_t = ld(vcb1, [G, 1])
            cw2_t = ld(cw2, [G, 1])
            cb2_t = ld(cb2, [1, 1])
            encw_t = ld(encw, [1, H])
            dstall_t = ld(dstf, [128, totcols], BF16)
            iotab_t = ld(iotab, [128, 128], BF16)
            nc.gpsimd.load_library(library_config.mlp)

            # ---- phase 1: hepsT[f, n] = (1+eps0) * (encw[f]*x[n] + encb[f])
            # (x and vheps pre-scaled by (1+eps0) on host)
            hepsT = bigp.tile([H, NPAD], BF16)
            for c0n in range(0, NPAD, 512):
                w = min(512, NPAD - c0n)
                x_t = sb.tile([1, 512], F32, tag="xchunk")
                nc.sync.dma_start(x_t[:, :w], xin[:, c0n : c0n + w])
                ps = psE.tile([H, 512], F32, space="PSUM", tag="mmE")
                nc.tensor.matmul(
                    out=ps[:, :w], lhsT=encw_t[:], rhs=x_t[:, :w],
                    start=True, stop=True,
                )
                nc.vector.tensor_tensor(
                    out=hepsT[:, c0n : c0n + w], in0=ps[:, :w],
                    in1=vheps_t[:].to_broadcast([H, w]), op=ADD,
                )

            # ---- phase 2: layer-1 message expansion + scatter (per supertile)
            zT = bigp.tile([H, NPAD], BF16)

            def scatter_tile(s, i, msg_t, oh_t, agg_pool):
                t = 2 * s + i
                agg = agg_pool.tile([H, 128], F32, space="PSUM", tag="agg")
                ntot = int(chunks[t])
                done = 0
                for r in range(4):
                    boff = int(cls_off2[s][r]) + (int(chcls[t - 1][r]) if i else 0)
                    for cc in range(int(chcls[t][r])):
                        col = boff + cc
                        done += 1
                        nc.tensor.matmul(
                            out=agg[:],
                            lhsT=msg_t[:, col, :],
                            rhs=oh_t[:, col, :],
                            start=(done == 1),
                            stop=(done == ntot),
                        )
                nc.vector.tensor_tensor(
                    out=zT[:, t * 128 : (t + 1) * 128],
                    in0=agg[:],
                    in1=hepsT[:, t * 128 : (t + 1) * 128],
                    op=ADD,
                )

            # ---- phase 3 (emitted interleaved): layer-1 GIN MLP; h1
            # feature-major (scaled by 1+eps1, overwrites hepsT) and
            # node-major bf16 rows (+ eenc bias) for the gather tables
            h1locq = []
            for q in range(4):
                h1q = dram.tile([int(QS[q]), H], BF16, name=f"h1locq{q}",
                                tag=f"h1locq{q}")
                h1locq.append(h1q)

            def mlp1_chunk(c0n):
                w = min(512, NPAD - c0n)
                ps1 = psE.tile([H, 512], F32, space="PSUM", tag="mmE",
                               name="ps1")
                nc.tensor.matmul(
                    out=ps1[:, :w], lhsT=w1l0_t[:], rhs=zT[:, c0n : c0n + w],
                    start=True, stop=True,
                )
                y1 = sb.tile([H, 512], BF16, tag="y1", name="y1")
                nc.scalar.activation(
                    out=y1[:, :w], in_=ps1[:, :w], func=RELU, bias=vb1l0_t[:]
                )
                ps2 = psE.tile([H, 512], F32, space="PSUM", tag="mmE",
                               name="ps2")
                nc.tensor.matmul(
                    out=ps2[:, :w], lhsT=w2l0_t[:], rhs=y1[:, :w],
                    start=True, stop=True,
                )
                nc.scalar.activation(
                    out=hepsT[:, c0n : c0n + w], in_=ps2[:, :w], func=RELU,
                    bias=vbnb0_t[:],
                )
                nc.vector.tensor_scalar(
                    out=hepsT[:, c0n : c0n + w], in0=hepsT[:, c0n : c0n + w],
                    scalar1=eps1p1, scalar2=None, op0=MULT,
                )
                for sub in range(w // 128):
                    n0 = c0n + sub * 128
                    ps2n = psN.tile([128, H], F32, space="PSUM", tag="mmN",
                                    name="ps2n")
                    nc.tensor.matmul(
                        out=ps2n[:],
                        lhsT=y1[:, sub * 128 : (sub + 1) * 128],
                        rhs=w2l0_t[:],
                        start=True, stop=True,
                    )
                    h1n = sb.tile([128, H], F32, tag="h1n", name="h1n")
                    nc.vector.tensor_tensor(
                        out=h1n[:], in0=ps2n[:], in1=repk_t[:, 0:128], op=ADD
                    )
                    nc.vector.tensor_scalar(
                        out=h1n[:], in0=h1n[:], scalar1=0.0, scalar2=None, op0=MAX
                    )
                    h1nb = sb.tile([128, H], BF16, tag="h1nb", name="h1nb")
                    nc.vector.tensor_tensor(
                        out=h1nb[:], in0=h1n[:], in1=repk_t[:, 256:384], op=ADD
                    )
                    q = int(np.searchsorted(QB[1:], n0, side="right"))
                    nc.sync.dma_start(
                        h1locq[q][n0 - int(QB[q]) : n0 - int(QB[q]) + 128, :],
                        h1nb[:],
                    )
                    if dbg:
                        nc.sync.dma_start(h1dbg[n0 : n0 + 128, :], h1nb[:])

            for s in range(NS):
                scol = int(chunks2[s])
                c0 = int(col_off2[s])
                g0 = int(sgrp_off[s])
                ngrp = int(ngrp2[s])
                p3_t = sb.tile([12, maxgrp2 * 128], BF16, tag="p3")
                nc.sync.dma_start(
                    p3_t[:, :], p3g[:, g0 * 128 : (g0 + maxgrp2) * 128]
                )
                msg_t = sb.tile([128, maxch2, H], BF16, tag="msg")
                for gi in range(ngrp):
                    ps = psE.tile([128, 512], F32, space="PSUM", tag="mmE")
                    nsub = min(4, scol - gi * 4)
                    nc.tensor.matmul(
                        out=ps[:],
                        lhsT=p3_t[:, gi * 128 : (gi + 1) * 128],
                        rhs=w3blk_t[:],
                        start=True, stop=True,
                    )
                    nc.scalar.activation(
                        out=msg_t[:, gi * 4 : gi * 4 + nsub, :],
                        in_=ps[:, : nsub * 128],
                        func=RELU,
                    )
                oh_t = sb.tile([128, maxch2, 128], BF16, tag="oh")
                nc.vector.tensor_tensor(
                    out=oh_t[:, :scol, :],
                    in0=dstall_t[:, c0 : c0 + scol].unsqueeze(2).to_broadcast([128, scol, 128]),
                    in1=iotab_t[:].unsqueeze(1).to_broadcast([128, scol, 128]),
                    op=ISEQ,
                )
                scatter_tile(s, 0, msg_t, oh_t, psA)
                scatter_tile(s, 1, msg_t, oh_t, psA)
                if s % 2 == 1:
                    mlp1_chunk((s // 2) * 512)
            mlp1_chunk(12288)

            # ---- phase 4: AllGather the h1 table as 4 quarter tables,
            # issued as each quarter's rows complete (overlaps phase 3)
            tq = []
            for q in range(4):
                qb, qs = int(QB[q]), int(QS[q])
                tq_q = dram.tile(
                    [NCORES * qs, H], BF16, addr_space="Shared",
                    name=f"tq{q}", tag=f"tq{q}",
                )
                nc.gpsimd.collective_compute(
                    "AllGather", mybir.AluOpType.bypass,
                    replica_groups=[list(range(NCORES))],
                    ins=[h1locq[q].opt()], outs=[tq_q.opt()],
                )
                tq.append(tq_q)

            # ---- phase 6 (emitted interleaved with phase 5):
            # layer-2 GIN MLP (node-major) + pooling
            pool_ps = psP.tile([H, G], F32, space="PSUM", tag="pool")

            def mlp2_chunk(c0n):
                w = min(512, NPAD - c0n)
                ps1 = psE.tile([H, 512], F32, space="PSUM", tag="mmE",
                               name="ps1b")
                nc.tensor.matmul(
                    out=ps1[:, :w], lhsT=w1l1_t[:], rhs=zT[:, c0n : c0n + w],
                    start=True, stop=True,
                )
                y1 = sb.tile([H, 512], BF16, tag="y1", name="y1b")
                nc.scalar.activation(
                    out=y1[:, :w], in_=ps1[:, :w], func=RELU, bias=vb1l1_t[:]
                )
                for sub in range(w // 128):
                    t = (c0n + sub * 128) // 128
                    ps2n = psN.tile([128, H], F32, space="PSUM", tag="mmN",
                                    name="ps2nb")
                    nc.tensor.matmul(
                        out=ps2n[:],
                        lhsT=y1[:, sub * 128 : (sub + 1) * 128],
                        rhs=w2l1_t[:],
                        start=True, stop=True,
                    )
                    h2n = sb.tile([128, H], F32, tag="h2n", name="h2n")
                    nc.vector.tensor_tensor(
                        out=h2n[:], in0=ps2n[:], in1=repk_t[:, 128:256], op=ADD
                    )
                    nc.vector.tensor_scalar(
                        out=h2n[:], in0=h2n[:], scalar1=0.0, scalar2=None, op0=MAX
                    )
                    gb_oh = sb.tile([128, G], F32, tag="gboh", name="gb_oh")
                    nc.vector.tensor_tensor(
                        out=gb_oh[:],
                        in0=repk_t[:, 576 + t : 577 + t].to_broadcast([128, G]),
                        in1=repk_t[:, 384:448],
                        op=ISEQ,
                    )
                    nc.tensor.matmul(
                        out=pool_ps[:],
                        lhsT=h2n[:],
                        rhs=gb_oh[:],
                        start=(t == 0),
                        stop=(t == NT - 1),
                    )

            # ---- phase 5: layer-2 gather + scatter (per supertile)
            nidx_regs = {}
            for s in range(NS):
                for r in range(4):
                    cq = int(chq[s][r])
                    if cq and cq not in nidx_regs:
                        nidx_regs[cq] = nc.gpsimd.to_reg(cq * 128)
            for s in range(NS):
                scol = int(chunks2[s])
                c0 = int(col_off2[s])
                g0 = int(sgrp_off[s])
                ngrp = int(ngrp2[s])
                idx_t = sb.tile([128, 8 * maxch2], I16, tag="idx16")
                nc.sync.dma_start(
                    idx_t[:, : 8 * scol], idx16[:, 8 * c0 : 8 * (c0 + scol)]
                )
                msg_t = sb.tile([128, maxch2, H], BF16, tag="msg")
                for r in range(4):
                    cq = int(chq[s][r])
                    if cq == 0:
                        continue
                    coff = int(cls_off2[s][r])
                    nc.gpsimd.dma_gather(
                        out_ap=msg_t[:, coff : coff + cq, :],
                        in_ap=tq[r][:, :],
                        idxs_ap=idx_t[:, 8 * coff : 8 * (coff + cq)],
                        num_idxs=cq * 128,
                        num_idxs_reg=nidx_regs[cq],
                        elem_size=H,
                        single_packet=False,
                        queue_num=r,
                    )
                p3_t = sb.tile([12, maxgrp2 * 128], BF16, tag="p3")
                nc.sync.dma_start(
                    p3_t[:, :], p3g[:, g0 * 128 : (g0 + maxgrp2) * 128]
                )
                eav_t = sb.tile([128, maxch2, H], BF16, tag="eav")
                for gi in range(ngrp):
                    ps = psE.tile([128, 512], F32, space="PSUM", tag="mmE")
                    nsub = min(4, scol - gi * 4)
                    nc.tensor.matmul(
                        out=ps[:],
                        lhsT=p3_t[:, gi * 128 : (gi + 1) * 128],
                        rhs=vblk12_t[:],
                        start=True, stop=True,
                    )
                    nc.scalar.activation(
                        out=eav_t[:, gi * 4 : gi * 4 + nsub, :],
                        in_=ps[:, : nsub * 128],
                        func=COPY,
                    )
                nc.vector.tensor_tensor(
                    out=msg_t[:, :scol, :], in0=msg_t[:, :scol, :],
                    in1=eav_t[:, :scol, :], op=ADD,
                )
                nc.vector.tensor_scalar(
                    out=msg_t[:, :scol, :], in0=msg_t[:, :scol, :],
                    scalar1=0.0, scalar2=None, op0=MAX,
                )
                oh_t = sb.tile([128, maxch2, 128], BF16, tag="oh")
                nc.vector.tensor_tensor(
                    out=oh_t[:, :scol, :],
                    in0=dstall_t[:, c0 : c0 + scol].unsqueeze(2).to_broadcast([128, scol, 128]),
                    in1=iotab_t[:].unsqueeze(1).to_broadcast([128, scol, 128]),
                    op=ISEQ,
                )
                scatter_tile(s, 0, msg_t, oh_t, psA)
                scatter_tile(s, 1, msg_t, oh_t, psA)
            for c0n in range(0, NPAD, 512):
                mlp2_chunk(c0n)

            # ---- phase 7: AllReduce pool partials + classifier
            poolbuf = sb.tile([H, G], F32, tag="poolo")
            nc.vector.tensor_copy(out=poolbuf[:], in_=pool_ps[:])
            poolin = dram.tile([H, G], F32)
            nc.sync.dma_start(poolin[:], poolbuf[:])
            if dbg:
                nc.sync.dma_start(pooldbg[:, :], poolbuf[:])
                nc.sync.dma_start(z2dbg[:, :], zT[:])
            poolsum = dram.tile([H, G], F32, addr_space="Shared")
            nc.gpsimd.collective_compute(
                "AllReduce", ADD,
                replica_groups=[list(range(NCORES))],
                ins=[poolin.opt()], outs=[poolsum.opt()],
            )
            pooled = sb.tile([H, G], F32, tag="pooled")
            nc.sync.dma_start(pooled[:], poolsum[:])
            nc.vector.tensor_tensor(
                out=pooled[:], in0=pooled[:], in1=repk_t[:, 512:576], op=MULT
            )
            ps_c1 = psN.tile([128, H], F32, space="PSUM", tag="mmN")
            nc.tensor.matmul(
                out=ps_c1[:G, :G], lhsT=cw1_t[:], rhs=pooled[:], start=True, stop=True
            )
            hc = sb.tile([G, G], F32, tag="hc")
            nc.scalar.activation(
                out=hc[:], in_=ps_c1[:G, :G], func=RELU, bias=vcb1_t[:]
            )
            ps_c2 = psN.tile([128, H], F32, space="PSUM", tag="mmN")
            nc.tensor.matmul(
                out=ps_c2[:1, :G], lhsT=cw2_t[:], rhs=hc[:], start=True, stop=True
            )
            ob = sb.tile([1, G], F32, tag="ob")
            nc.scalar.activation(out=ob[:], in_=ps_c2[:1, :G], func=SIG, bias=cb2_t[:])
            nc.sync.dma_start(out_o[:, :], ob[:])
    return nc


# ---------------------------------------------------------------- driver


def kernel(x, edge_index, edge_attr, batch,
           enc_w, enc_b, eenc_w, eenc_b,
           eps, w1, b1, w2, b2, gamma, beta,
           cw1, cb1, cw2, cb2):
    _install_fixups()
    x = np.asarray(x, np.float32).reshape(-1)
    src = np.asarray(edge_index[0], np.int64)
    dst = np.asarray(edge_index[1], np.int64)
    ea = np.asarray(edge_attr, np.float32).reshape(-1)
    batch = np.asarray(batch, np.int64)
    inv_std = np.float32(1.0 / np.sqrt(1.0 + BN_EPS))
    cores = list(range(NCORES))

    (order, core_s, tile_s, cls_s, dloc_s, starts, chunks, chcls, chq,
     cls_off2, col_off2, ngrp2, sgrp_off, totcols, totgrp) = _edge_meta(src, dst)

    u = np.asarray(enc_w, np.float32).reshape(H)
    v = np.asarray(eenc_w, np.float32).reshape(H)
    b0 = np.asarray(enc_b, np.float32).reshape(H)
    be = np.asarray(eenc_b, np.float32).reshape(H)
    iota_np = np.tile(np.arange(128, dtype=np.float32)[None, :], (128, 1))
    eps_np = np.asarray(eps, np.float32)

    def bn_fold(l):
        g = np.asarray(gamma, np.float32)[l]
        s = g * inv_std
        bvec = np.asarray(beta, np.float32)[l] + s * np.asarray(b2, np.float32)[l]
        return s, bvec

    bns0, bnb0 = bn_fold(0)
    bns1, bnb1 = bn_fold(1)
    w1_np = np.asarray(w1, np.float32)
    w2_np = np.asarray(w2, np.float32)
    b1_np = np.asarray(b1, np.float32)
    w2l0 = w2_np[0] * bns0[None, :]
    w2l1 = w2_np[1] * bns1[None, :]

    cnt = np.bincount(batch, minlength=G).astype(np.float32)
    invg = (1.0 / np.maximum(cnt, 1.0)).astype(np.float32)

    maxgrp2 = max(int(ngrp2.max()), 1)

    nc = _build(chunks, chcls, chq, cls_off2, col_off2, ngrp2, sgrp_off,
                totcols, totgrp, float(1.0 + eps_np[1]))
    slot_pc = _edge_fill(
        src, ea, x, order, core_s, tile_s, cls_s, dloc_s, starts,
        chcls, cls_off2, col_off2, sgrp_off, totcols, totgrp, maxgrp2,
    )

    # block-diag expansion weights: rows 3j+r carry w3 row r in block j
    w3blk_np = np.zeros((12, 512), np.float32)
    vblk12_np = np.zeros((12, 512), np.float32)
    w3f = np.stack([u, v, b0 + be])
    for j in range(4):
        for r in range(3):
            w3blk_np[3 * j + r, 128 * j : 128 * (j + 1)] = w3f[r]
        vblk12_np[3 * j + 1, 128 * j : 128 * (j + 1)] = v


    in_maps = []
    for k in cores:
        pc = slot_pc[k]
        xloc = np.zeros(NPAD, np.float32)
        xloc[:NLOC] = x[k * NLOC : (k + 1) * NLOC]
        gloc = np.full(NPAD, -1.0, np.float32)
        gloc[:NLOC] = batch[k * NLOC : (k + 1) * NLOC].astype(np.float32)
        in_maps.append({
            "p3g": pc["p3g"],
            "dstf": pc["sloc"],
            "idx16": pc["idx16"],
            "x": (xloc * (1.0 + eps_np[0])).reshape(1, NPAD),
            "encw": u.reshape(1, H),
            "w3blk": w3blk_np.astype(bf),
            "vblk12": vblk12_np.astype(bf),
            "iotab": iota_np.astype(bf),
            "w1l0": w1_np[0].astype(bf), "w2l0": w2l0.astype(bf),
            "w1l1": w1_np[1].astype(bf), "w2l1": w2l1.astype(bf),
            "vb1l0": b1_np[0].reshape(H, 1),
            "vb1l1": b1_np[1].reshape(H, 1),
            "vheps": ((1.0 + eps_np[0]) * b0).reshape(H, 1),
            "vbnb0": bnb0.reshape(H, 1),
            "repk": np.concatenate([
                np.tile(bnb0[None, :], (128, 1)),
                np.tile(bnb1[None, :], (128, 1)),
                np.tile(be[None, :], (128, 1)),
                iota_np,
                np.tile(invg[None, :], (128, 1)),
                np.ascontiguousarray(gloc.reshape(NT, 128).T),
            ], axis=1).astype(np.float32),
            "cw1": np.asarray(cw1, np.float32),
            "vcb1": np.asarray(cb1, np.float32).reshape(G, 1),
            "cw2": np.asarray(cw2, np.float32),
            "cb2": np.asarray(cb2, np.float32).reshape(1, 1),
        })

    library_overlay.lower_extended_insts(nc)
    res = run_bass_kernel_spmd(nc, in_maps, cores)
    kernel.last_results = res
    return res.results[0]["out"].reshape(G).astype(np.float32)

